# revision 20
# baseline (speedup 1.0000x reference)
"""Causal multi-head attention block (B=2, T=2048, C=1024, H=16) on 8 TRN2
NeuronCores.

Sharding: Megatron-style tensor parallel over heads for QKV + attention
(core r owns heads {2r, 2r+1} = feature rows [128r, 128r+128) of the
attention output), then a token-sharded output projection: cores
exchange attention outputs with ONE AllToAll per batch entry (each core
sends its 128 feature rows of everyone's 256-token block, receives the
full 1024 feature rows of its own 256-token block), and each core
computes out[:, its tokens] = Wo @ y_full with the full Wo replicated.

v2 used Megatron all-gather + column-sharded O-projection: that moves
8 MB per core through the collective stream (AllGather replicates), and
the profiled CC stream (45us first-collective barrier + 8 x ~20us
sequential RDH AllGathers at ~50 GB/s) was the critical path of the
whole second half. The AllToAll moves only ~0.5 MB per core per batch
(mesh algorithm, ~10us) and there are just two of them.

Everything on-device is computed in the "transposed" orientation
(feature-major, token-minor) so the TensorEngine contraction axis always
sits on SBUF partitions and the softmax denominator arrives for free via
a ones-column appended to V:

  qT/kT/vT [128, 4096] = W_shard @ x^T          (x^T passed from host)
  ST tile [128k, 512q] = kT_slice.T @ qT_slice  (contract d=64)
  causal mask: add a -1e9 strictly-lower-triangular matrix into the St
      PSUM accumulation group via matmul(ident, mneg) on diagonal blocks
  PT = exp(ST * 1/sqrt(d))                      (no max-subtraction: logits
                                                 are ~N(0,1), |S|max ~ 6)
  yT [65, 512] += [v | 1].T @ PT                (row 64 = softmax denom)
  yT_norm = yT[0:64] * partition_broadcast(recip(yT[64]))

Performance structure (from perfetto/HAM/cc_ops analysis of v1/v2):
- The attention is a flat list of 80 "pair units" (2 k-tiles sharing a
  2-bank PSUM st tile, one 1024-wide EXP each), software-pipelined:
  scores(i+1) are program-ordered BEFORE pv(i) so the in-order PE queue
  never waits on the ACT exp of the current pair.
- All projections except tch0, and the b0 O-projection, are filler
  thunks injected between units at fixed slots, keeping the PE stream
  dense (HAM stays un-throttled) while ACT runs exps back to back.
- A tiny dummy AllGather fires at t~0 so the ~45us one-time ncfw
  barrier (paid at the first collective) overlaps the prologue instead
  of the first real collective consumer.
- xT is loaded with 4 big dma_starts (each fans out over all 16 DMA
  engines) on 3 queues; the scalar queue only carries the 80 exps plus
  the tch0 load, so DMA issue never delays them. The exp table set is
  pre-loaded with a dummy activation at t=0.

Inputs are bf16 (host-side cast); accumulation is f32 in PSUM; the output
shard is written bf16 and upcast to f32 on the host.
"""

import numpy as np
import ml_dtypes

import concourse.bacc as bacc
import concourse.mybir as mybir
import concourse.tile as tile
from concourse.bass_utils import run_bass_kernel_spmd
from concourse.masks import make_identity

N_CORES = 8
B, T, C, H = 2, 2048, 1024, 16
D = 64                # head dim
HL = H // N_CORES     # heads per core = 2
DL = HL * D           # local feature dim = 128
TT = B * T            # 4096 tokens total
P = 128
NCH = C // P          # 8 contraction chunks
QCH = 512             # q-chunk (moving free dim)
NQC = T // QCH        # 4 q-chunks per batch entry
NKT = T // P          # 16 k-tiles per batch entry
TTOK = T // N_CORES   # 256: tokens per core per batch in the A2A split
SCALE = 1.0 / np.sqrt(D)

BF = mybir.dt.bfloat16
F32 = mybir.dt.float32
AF = mybir.ActivationFunctionType

W_QKV = 3 * NCH * DL          # 3072 cols of packed qkv shards
W_O = NCH * C                 # 8192 cols of packed full Wo


def build_graph():
    nc = bacc.Bacc("TRN2", target_bir_lowering=False, debug=False)

    xT = nc.dram_tensor("xT", [C, TT], BF, kind="ExternalInput")
    wall = nc.dram_tensor("wall", [P, W_QKV + W_O], BF, kind="ExternalInput")
    # out[:, 0:256] = batch-0 tokens [256r, 256r+256); [:, 256:512] same
    # for batch 1
    out = nc.dram_tensor("out", [C, 2 * TTOK], BF, kind="ExternalOutput")

    with tile.TileContext(nc) as tc:
        with (
            tc.tile_pool(name="sb", bufs=1) as sb,
            tc.tile_pool(name="ps", bufs=1, space="PSUM") as ps,
            tc.tile_pool(name="dram", bufs=1, space="DRAM") as dram,
        ):
            # ---- collective warm-up: absorb the one-time ncfw barrier and
            # the first-AllToAll setup cost ----
            ccw_in = dram.tile([N_CORES, 32], BF, name="ccw_in")
            ccw_out = dram.tile([N_CORES, 32], BF, name="ccw_out")
            nc.gpsimd.collective_compute(
                "AllToAll", mybir.AluOpType.bypass,
                replica_groups=[list(range(N_CORES))],
                ins=[ccw_in[:]], outs=[ccw_out[:]],
            )

            # ---- loads ----
            # xT chunk [512:1024] leads the gpsimd queue so tch1 doesn't
            # serialize behind the big weight transfer on sync
            xT_sb = sb.tile([P, NCH, TT], BF, name="xT_sb")
            xTr = xT[:].rearrange("(a p) t -> p a t", p=P)
            nc.gpsimd.dma_start(xT_sb[:, :, QCH:2 * QCH],
                                xTr[:, :, QCH:2 * QCH])
            nc.gpsimd.dma_start(xT_sb[:, :, 4 * QCH:TT], xTr[:, :, 4 * QCH:TT])

            w_sb = sb.tile([P, W_QKV + W_O], BF, name="w_sb")
            nc.sync.dma_start(w_sb[:, 0:W_QKV], wall[:, 0:W_QKV])
            w3 = w_sb[:, 0:W_QKV].rearrange("p (w a m) -> p w a m",
                                            w=3, a=NCH)
            wq_sb, wk_sb, wv_sb = (w3[:, i] for i in range(3))
            # full Wo^T packed as [p, ci, ro, m]:
            # Wo.T[ci*128+p, ro*128+m]
            wo_sb = w_sb[:, W_QKV:].rearrange("p (a r m) -> p a r m",
                                              a=NCH, r=NCH)

            ident = sb.tile([P, P], BF, name="ident")
            make_identity(nc, ident)
            # strictly-lower-triangular -1e9 (k > q): masks causal logits on
            # diagonal blocks, injected into the St PSUM group via
            # matmul(ident, mneg)
            mneg = sb.tile([P, P], BF, name="mneg")
            nc.gpsimd.memset(mneg[:], 0.0)
            nc.gpsimd.affine_select(
                out=mneg[:], in_=mneg[:],
                compare_op=mybir.AluOpType.is_ge,
                fill=-1e9, base=0, channel_multiplier=-1, pattern=[[1, P]],
            )
            wsrc = sb.tile([P, QCH], BF, name="wsrc")
            nc.vector.memset(wsrc[:], 0.5)
            # K=1 stationary of ones: broadcasts the denominator row across
            # 64 partitions on the (otherwise idle-at-that-moment) PE
            ones_col = sb.tile([1, D], F32, name="ones_col")
            nc.vector.memset(ones_col[:], 1.0)
            # pre-load the exp table set so the first real exp doesn't pay
            # the ~2.7us ACT_TABLE_LOAD on the critical path
            expwarm = sb.tile([P, 32], F32, name="expwarm")
            nc.scalar.activation(expwarm[:], wsrc[:, 0:32], AF.Exp, scale=1.0)

            # remaining xT + the big Wo block (needed only mid-kernel)
            nc.scalar.dma_start(xT_sb[:, :, 0:QCH], xTr[:, :, 0:QCH])
            nc.sync.dma_start(xT_sb[:, :, 2 * QCH:4 * QCH],
                              xTr[:, :, 2 * QCH:4 * QCH])
            nc.sync.dma_start(w_sb[:, W_QKV:], wall[:, W_QKV:])

            qT_sb = sb.tile([P, TT], BF, name="qT_sb")
            kT_sb = sb.tile([P, TT], BF, name="kT_sb")
            vT_sb = sb.tile([P, TT], BF, name="vT_sb")
            # v in natural layout, packed per 128-token tile as
            # [headA(64) | 1 | headB(64) | 1] -> 130 columns
            v_sb = sb.tile([P, TT // P, 2 * (D + 1)], BF, name="v_sb")
            nc.gpsimd.memset(v_sb[:], 1.0)

            # warmup matmuls: cover the initial DMA window + HAM ramp
            for _ in range(10):
                wdst = ps.tile([P, QCH], F32, tag="fill", bufs=2, name="wdst")
                nc.tensor.matmul(wdst[:], ident[:], wsrc[:],
                                 start=True, stop=True)

            # ---- AllToAll plumbing: one exchange per (batch, half) ----
            # Exchange e=(b,hf) covers b's tokens [hf*1024, hf*1024+1024) in
            # 8 blocks of TOKB=128. in[e][s] = my 128 feature rows of block
            # s; out[e][s] = rank s's 128 feature rows of MY block.
            TOKB = T // 2 // N_CORES  # 128
            a2a_in = {(b, hf): dram.tile([N_CORES, P, TOKB], BF,
                                         name=f"a2a_in{b}{hf}")
                      for b in range(B) for hf in range(2)}
            a2a_out = {(b, hf): dram.tile([N_CORES, P, TOKB], BF,
                                          name=f"a2a_out{b}{hf}")
                       for b in range(B) for hf in range(2)}

            def proj_group(tch, wsb, dst):
                tsl = slice(tch * QCH, (tch + 1) * QCH)
                pj = ps.tile([P, QCH], F32, tag="fill", bufs=2, name="pj")
                for ci in range(NCH):
                    nc.tensor.matmul(
                        pj[:], wsb[:, ci, :], xT_sb[:, ci, tsl],
                        start=(ci == 0), stop=(ci == NCH - 1),
                    )
                nc.vector.tensor_copy(dst[:, tsl], pj[:])

            def vtrans(t32):
                tr = ps.tile([P, P], BF, tag="fill", bufs=2, name="tr")
                nc.tensor.transpose(
                    tr[:], vT_sb[:, t32 * P:(t32 + 1) * P], ident[:]
                )
                out_ap = v_sb[:, t32, :].rearrange(
                    "p (h x) -> p h x", h=HL
                )[:, :, 0:D]
                in_ap = tr[:].rearrange("p (h x) -> p h x", h=HL)
                nc.vector.tensor_copy(out_ap, in_ap)

            ya_tiles = {}
            ob_tiles = {}

            def ya_load(b, hf):
                if b not in ya_tiles:
                    ya_tiles[b] = sb.tile([P, NCH, TTOK], BF, tag="ya",
                                          bufs=2, name="ya")
                    ob_tiles[b] = sb.tile([P, NCH, TTOK], BF, tag="ob",
                                          bufs=2, name="ob")
                nc.sync.dma_start(
                    ya_tiles[b][:, :, hf * TOKB:(hf + 1) * TOKB],
                    a2a_out[(b, hf)][:].rearrange("a p t -> p a t"),
                )

            def po_ro(b, ro, hf):
                # half-split so the half-0 matmuls can run while the second
                # A2A of the batch is still in flight
                ya = ya_tiles[b]
                tsl = slice(hf * TOKB, (hf + 1) * TOKB)
                po = ps.tile([P, TOKB], F32, tag="fill", bufs=2, name="po")
                for ci in range(NCH):
                    nc.tensor.matmul(
                        po[:], wo_sb[:, ci, ro, :], ya[:, ci, tsl],
                        start=(ci == 0), stop=(ci == NCH - 1),
                    )
                ob = ob_tiles[b]
                nc.vector.tensor_copy(ob[:, ro, tsl], po[:])
                if hf == 1:
                    # per-ro output write overlaps the next ro's matmuls
                    ov = out[:].rearrange("(r p) t -> p r t", p=P)
                    nc.sync.dma_start(
                        ov[:, ro, b * TTOK:(b + 1) * TTOK], ob[:, ro, :]
                    )

            # ---- prologue: projections for tch0 only ----
            for wsb, dst in ((wq_sb, qT_sb), (wk_sb, kT_sb), (wv_sb, vT_sb)):
                proj_group(0, wsb, dst)
            for t32 in range(4):
                vtrans(t32)

            # ---- the 80 attention pair-units ----
            units = []
            for b in range(B):
                for jq in range(NQC):
                    nkt = 4 * jq + 4
                    for h in range(HL):
                        for pr in range(nkt // 2):
                            units.append((b, jq, h, pr, nkt))
            NU = len(units)

            # filler schedule: unit index -> list of thunks
            SCHED = {}

            def at(u, fn, *args):
                SCHED.setdefault(u, []).append((fn, args))

            # projections tch1..7 (tch_k needed by (b=k//4, jq=k%4))
            proj_slots = {1: 0, 2: 4, 3: 12, 4: 20, 5: 27, 6: 33, 7: 44}
            for tch, u0 in proj_slots.items():
                for k, (wsb, dst) in enumerate(((wq_sb, qT_sb),
                                               (wk_sb, kT_sb),
                                               (wv_sb, vT_sb))):
                    at(u0 + k, proj_group, tch, wsb, dst)
                at(u0 + 3, vtrans, tch * 4 + 0)
                at(u0 + 3, vtrans, tch * 4 + 1)
                at(u0 + 4, vtrans, tch * 4 + 2)
                at(u0 + 4, vtrans, tch * 4 + 3)

            # b0's A2As fire at units 11 and 39; b1's first at unit 51.
            # Consumers are scheduled LATE: the first exchange can't finish
            # before the ~60us ncfw barrier, and inter-core skew (measured
            # up to ~40us) delays completion further — a too-early consumer
            # stalls this core's whole in-order PE queue.
            at(46, ya_load, 0, 0)
            for ro in range(NCH):
                at(56 + ro, po_ro, 0, ro, 0)
            at(64, ya_load, 0, 1)
            for ro in range(NCH):
                at(68 + ro, po_ro, 0, ro, 1)
            at(72, ya_load, 1, 0)

            st_of = {}
            pt_of = {}
            yt_of = {}
            den_of = {}

            def sc_issue(i):
                b, jq, h, pr, nkt = units[i]
                rsl = slice(h * D, (h + 1) * D)
                q0 = b * T + jq * QCH
                st = ps.tile([P, 2 * QCH], F32, tag="st", bufs=2, name="st")
                st_of[i] = st
                diag = []
                for half in range(2):
                    kt = 2 * pr + half
                    k0 = b * T + kt * P
                    ii = kt - 4 * jq
                    qv = max(ii, 0) * P
                    ssl = slice(half * QCH + qv, (half + 1) * QCH)
                    nc.tensor.matmul(
                        st[:, ssl],
                        kT_sb[rsl, k0:k0 + P],
                        qT_sb[rsl, q0 + qv:q0 + QCH],
                        start=True, stop=(ii < 0),
                    )
                    if ii >= 0:
                        diag.append(half * QCH + qv)
                for cq in diag:
                    nc.tensor.matmul(
                        st[:, cq:cq + P], ident[:], mneg[:],
                        start=False, stop=True,
                    )

            def exp_issue(i):
                b, jq, h, pr, nkt = units[i]
                qv0 = max(2 * pr - 4 * jq, 0) * P
                pt = sb.tile([P, 2 * QCH], BF, tag="pt", bufs=4, name="pt")
                pt_of[i] = pt
                nc.scalar.activation(
                    pt[:, qv0:], st_of[i][:, qv0:], AF.Exp,
                    scale=float(SCALE)
                )

            def pv_issue(i):
                b, jq, h, pr, nkt = units[i]
                if pr == 0:
                    yt_of[(b, jq, h)] = ps.tile([D + 1, QCH], F32, tag="yt",
                                                bufs=2, name="yt")
                yt = yt_of[(b, jq, h)]
                pt = pt_of.pop(i)
                for half in range(2):
                    kt = 2 * pr + half
                    qv = max(kt - 4 * jq, 0) * P
                    nc.tensor.matmul(
                        yt[:, qv:QCH],
                        v_sb[:, b * NKT + kt,
                             h * (D + 1):(h + 1) * (D + 1)],
                        pt[:, half * QCH + qv:(half + 1) * QCH],
                        start=(kt == 0), stop=(kt == nkt - 1),
                    )
                st_of.pop(i)
                if pr == nkt // 2 - 1:
                    den = sb.tile([1, QCH], F32, tag="den", bufs=4,
                                  name="den")
                    nc.vector.tensor_copy(den[:], yt[D:D + 1, :])
                    den_of[(b, jq, h)] = den

            def evict(b, jq, h):
                yt = yt_of.pop((b, jq, h))
                den = den_of.pop((b, jq, h))
                bc = ps.tile([D, QCH], F32, tag="fill", bufs=2, name="bc")
                nc.tensor.matmul(bc[:], ones_col[:], den[:],
                                 start=True, stop=True)
                rcp = sb.tile([D, QCH], F32, tag="rcp", bufs=2, name="rcp")
                nc.vector.reciprocal_approx_fast(rcp[:], bc[:])
                yn = sb.tile([D, QCH], BF, tag="yn", bufs=4, name="yn")
                nc.vector.tensor_mul(yn[:], yt[0:D, :], rcp[:])
                # tokens [jq*512, jq*512+512) = blocks [4*(jq%2), +4) of
                # exchange (b, jq//2). SBUF side keeps partition dim first;
                # the DRAM side view is reordered to match.
                hf = jq // 2
                s0 = 4 * (jq % 2)
                nc.gpsimd.dma_start(
                    a2a_in[(b, hf)][s0:s0 + 4, h * D:(h + 1) * D, :]
                    .rearrange("s d t -> d s t"),
                    yn[:].rearrange("d (s t) -> d s t", s=4),
                )
                if jq % 2 == 1 and h == HL - 1:
                    nc.gpsimd.collective_compute(
                        "AllToAll", mybir.AluOpType.bypass,
                        replica_groups=[list(range(N_CORES))],
                        ins=[a2a_in[(b, hf)][:]], outs=[a2a_out[(b, hf)][:]],
                    )
                return yn

            # ---- pipelined emission ----
            sc_issue(0)
            exp_issue(0)
            last_yn = None
            for i in range(NU):
                if i + 1 < NU:
                    sc_issue(i + 1)
                    exp_issue(i + 1)
                pv_issue(i)
                b, jq, h, pr, nkt = units[i]
                if pr == nkt // 2 - 1:
                    last_yn = evict(b, jq, h)
                for fn, args in SCHED.get(i, ()):
                    fn(*args)

            # ---- tail: b1's last A2A + O-projection ----
            # the half-0 projections (data arrived with b1's first A2A)
            # bridge the final A2A's latency with useful PE work; a few
            # anchored warm matmuls (reading the last yn so Tile can't
            # hoist them) cover the rest
            ya_load(1, 1)
            for ro in range(NCH):
                po_ro(1, ro, 0)
            for _ in range(6):
                wdst = ps.tile([P, QCH], F32, tag="fill", bufs=2, name="wdst")
                nc.tensor.matmul(wdst[:], ident[0:D, :], last_yn[:],
                                 start=True, stop=True)
            for ro in range(NCH):
                po_ro(1, ro, 1)

    nc.finalize()
    return nc


_GRAPH = None


def _get_graph():
    global _GRAPH
    if _GRAPH is None:
        _GRAPH = build_graph()
    return _GRAPH


def prepare_in_maps(x, Wq, Wk, Wv, Wo):
    x = np.asarray(x, np.float32)
    Wq = np.asarray(Wq, np.float32)
    Wk = np.asarray(Wk, np.float32)
    Wv = np.asarray(Wv, np.float32)
    Wo = np.asarray(Wo, np.float32)

    bf = ml_dtypes.bfloat16
    xTh = np.ascontiguousarray(x.reshape(TT, C).T).astype(bf)
    # full Wo^T packed as [p, ci, ro, m] = Wo.T[ci*128+p, ro*128+m]
    wo_pack = Wo.T.reshape(NCH, P, NCH, P).transpose(1, 0, 2, 3)
    in_maps = []
    for r in range(N_CORES):
        sl = slice(r * DL, (r + 1) * DL)
        # pack the 3 transposed qkv weight shards into the SBUF layout
        # [p, w, ci, m] where the shard row index is c = ci*128 + p
        wqkv = np.empty((P, 3, NCH, DL), np.float32)
        for w, W in enumerate((Wq, Wk, Wv)):
            wqkv[:, w] = W[sl].T.reshape(NCH, P, DL).transpose(1, 0, 2)
        wall = np.concatenate(
            [wqkv.reshape(P, W_QKV), wo_pack.reshape(P, W_O)], axis=1
        )
        in_maps.append({
            "xT": xTh,
            "wall": np.ascontiguousarray(wall).astype(bf),
        })
    return in_maps


def assemble_output(results):
    TOKB = T // 2 // N_CORES
    outT = np.empty((C, TT), np.float32)
    for r in range(N_CORES):
        o = np.asarray(results[r]["out"], np.float32)  # [C, 2*TTOK]
        for b in range(B):
            ob = o[:, b * TTOK:(b + 1) * TTOK]
            for hf in range(2):
                t0 = b * T + hf * (T // 2) + r * TOKB
                outT[:, t0:t0 + TOKB] = ob[:, hf * TOKB:(hf + 1) * TOKB]
    return np.ascontiguousarray(outT.T).reshape(B, T, C)


def kernel(x, Wq, Wk, Wv, Wo):
    nc = _get_graph()
    in_maps = prepare_in_maps(x, Wq, Wk, Wv, Wo)
    res = run_bass_kernel_spmd(nc, in_maps, core_ids=list(range(N_CORES)))
    return assemble_output(res.results)


# revision 21
# speedup vs baseline: 1.2976x; 1.2976x over previous
"""Causal multi-head attention block (B=2, T=2048, C=1024, H=16) on 8 TRN2
NeuronCores.

Sharding: Megatron-style tensor parallel over heads for QKV + attention
(core r owns heads {2r, 2r+1} = feature rows [128r, 128r+128) of the
attention output), then a token-sharded output projection: cores
exchange attention outputs with ONE AllToAll per batch entry (each core
sends its 128 feature rows of everyone's 256-token block, receives the
full 1024 feature rows of its own 256-token block), and each core
computes out[:, its tokens] = Wo @ y_full with the full Wo replicated.

v2 used Megatron all-gather + column-sharded O-projection: that moves
8 MB per core through the collective stream (AllGather replicates), and
the profiled CC stream (45us first-collective barrier + 8 x ~20us
sequential RDH AllGathers at ~50 GB/s) was the critical path of the
whole second half. The AllToAll moves only ~0.5 MB per core per batch
(mesh algorithm, ~10us) and there are just two of them.

Everything on-device is computed in the "transposed" orientation
(feature-major, token-minor) so the TensorEngine contraction axis always
sits on SBUF partitions and the softmax denominator arrives for free via
a ones-column appended to V:

  qT/kT/vT [128, 4096] = W_shard @ x^T          (x^T passed from host)
  ST tile [128k, 512q] = kT_slice.T @ qT_slice  (contract d=64)
  causal mask: add a -1e9 strictly-lower-triangular matrix into the St
      PSUM accumulation group via matmul(ident, mneg) on diagonal blocks
  PT = exp(ST * 1/sqrt(d))                      (no max-subtraction: logits
                                                 are ~N(0,1), |S|max ~ 6)
  yT [65, 512] += [v | 1].T @ PT                (row 64 = softmax denom)
  yT_norm = yT[0:64] * partition_broadcast(recip(yT[64]))

Performance structure (from perfetto/HAM/cc_ops analysis of v1/v2):
- The attention is a flat list of 80 "pair units" (2 k-tiles sharing a
  2-bank PSUM st tile, one 1024-wide EXP each), software-pipelined:
  scores(i+1) are program-ordered BEFORE pv(i) so the in-order PE queue
  never waits on the ACT exp of the current pair.
- All projections except tch0, and the b0 O-projection, are filler
  thunks injected between units at fixed slots, keeping the PE stream
  dense (HAM stays un-throttled) while ACT runs exps back to back.
- A tiny dummy AllGather fires at t~0 so the ~45us one-time ncfw
  barrier (paid at the first collective) overlaps the prologue instead
  of the first real collective consumer.
- xT is loaded with 4 big dma_starts (each fans out over all 16 DMA
  engines) on 3 queues; the scalar queue only carries the 80 exps plus
  the tch0 load, so DMA issue never delays them. The exp table set is
  pre-loaded with a dummy activation at t=0.

Inputs are bf16 (host-side cast); accumulation is f32 in PSUM; the output
shard is written bf16 and upcast to f32 on the host.
"""

import numpy as np
import ml_dtypes

import concourse.bacc as bacc
import concourse.mybir as mybir
import concourse.tile as tile
from concourse.bass_utils import run_bass_kernel_spmd
from concourse.masks import make_identity

N_CORES = 8
B, T, C, H = 2, 2048, 1024, 16
D = 64                # head dim
HL = H // N_CORES     # heads per core = 2
DL = HL * D           # local feature dim = 128
TT = B * T            # 4096 tokens total
P = 128
NCH = C // P          # 8 contraction chunks
QCH = 512             # q-chunk (moving free dim)
NQC = T // QCH        # 4 q-chunks per batch entry
NKT = T // P          # 16 k-tiles per batch entry
TTOK = T // N_CORES   # 256: tokens per core per batch in the A2A split
SCALE = 1.0 / np.sqrt(D)

BF = mybir.dt.bfloat16
F32 = mybir.dt.float32
AF = mybir.ActivationFunctionType

W_QKV = 3 * NCH * DL          # 3072 cols of packed qkv shards
W_O = NCH * C                 # 8192 cols of packed full Wo


def build_graph():
    nc = bacc.Bacc("TRN2", target_bir_lowering=False, debug=False)

    xT = nc.dram_tensor("xT", [C, TT], BF, kind="ExternalInput")
    wall = nc.dram_tensor("wall", [P, W_QKV + W_O], BF, kind="ExternalInput")
    # out[:, 0:256] = batch-0 tokens [256r, 256r+256); [:, 256:512] same
    # for batch 1
    out = nc.dram_tensor("out", [C, 2 * TTOK], BF, kind="ExternalOutput")

    with tile.TileContext(nc) as tc:
        with (
            tc.tile_pool(name="sb", bufs=1) as sb,
            tc.tile_pool(name="ps", bufs=1, space="PSUM") as ps,
            tc.tile_pool(name="dram", bufs=1, space="DRAM") as dram,
        ):
            # ---- collective warm-up: absorb the one-time ncfw barrier and
            # the first-AllToAll setup cost ----
            ccw_in = dram.tile([N_CORES, 32], BF, name="ccw_in")
            ccw_out = dram.tile([N_CORES, 32], BF, name="ccw_out")
            nc.gpsimd.collective_compute(
                "AllToAll", mybir.AluOpType.bypass,
                replica_groups=[list(range(N_CORES))],
                ins=[ccw_in[:]], outs=[ccw_out[:]],
            )

            # ---- loads ----
            # xT chunk [512:1024] leads the gpsimd queue so tch1 doesn't
            # serialize behind the big weight transfer on sync
            xT_sb = sb.tile([P, NCH, TT], BF, name="xT_sb")
            xTr = xT[:].rearrange("(a p) t -> p a t", p=P)
            nc.gpsimd.dma_start(xT_sb[:, :, QCH:2 * QCH],
                                xTr[:, :, QCH:2 * QCH])
            nc.gpsimd.dma_start(xT_sb[:, :, 4 * QCH:TT], xTr[:, :, 4 * QCH:TT])

            w_sb = sb.tile([P, W_QKV + W_O], BF, name="w_sb")
            nc.sync.dma_start(w_sb[:, 0:W_QKV], wall[:, 0:W_QKV])
            w3 = w_sb[:, 0:W_QKV].rearrange("p (w a m) -> p w a m",
                                            w=3, a=NCH)
            wq_sb, wk_sb, wv_sb = (w3[:, i] for i in range(3))
            # full Wo^T packed as [p, ci, ro, m]:
            # Wo.T[ci*128+p, ro*128+m]
            wo_sb = w_sb[:, W_QKV:].rearrange("p (a r m) -> p a r m",
                                              a=NCH, r=NCH)

            ident = sb.tile([P, P], BF, name="ident")
            make_identity(nc, ident)
            # strictly-lower-triangular -1e9 (k > q): masks causal logits on
            # diagonal blocks, injected into the St PSUM group via
            # matmul(ident, mneg)
            mneg = sb.tile([P, P], BF, name="mneg")
            nc.gpsimd.memset(mneg[:], 0.0)
            nc.gpsimd.affine_select(
                out=mneg[:], in_=mneg[:],
                compare_op=mybir.AluOpType.is_ge,
                fill=-1e9, base=0, channel_multiplier=-1, pattern=[[1, P]],
            )
            wsrc = sb.tile([P, QCH], BF, name="wsrc")
            nc.vector.memset(wsrc[:], 0.5)
            # K=1 stationary of ones: broadcasts the denominator row across
            # 64 partitions on the (otherwise idle-at-that-moment) PE
            ones_col = sb.tile([1, D], F32, name="ones_col")
            nc.vector.memset(ones_col[:], 1.0)
            # pre-load the exp table set so the first real exp doesn't pay
            # the ~2.7us ACT_TABLE_LOAD on the critical path
            expwarm = sb.tile([P, 32], F32, name="expwarm")
            nc.scalar.activation(expwarm[:], wsrc[:, 0:32], AF.Exp, scale=1.0)

            # remaining xT + the big Wo block (needed only mid-kernel)
            nc.scalar.dma_start(xT_sb[:, :, 0:QCH], xTr[:, :, 0:QCH])
            nc.sync.dma_start(xT_sb[:, :, 2 * QCH:4 * QCH],
                              xTr[:, :, 2 * QCH:4 * QCH])
            nc.sync.dma_start(w_sb[:, W_QKV:], wall[:, W_QKV:])

            qT_sb = sb.tile([P, TT], BF, name="qT_sb")
            kT_sb = sb.tile([P, TT], BF, name="kT_sb")
            vT_sb = sb.tile([P, TT], BF, name="vT_sb")
            # v in natural layout, packed per 128-token tile as
            # [headA(64) | 1 | headB(64) | 1] -> 130 columns
            v_sb = sb.tile([P, TT // P, 2 * (D + 1)], BF, name="v_sb")
            nc.gpsimd.memset(v_sb[:], 1.0)

            # warmup matmuls: cover the initial DMA window + HAM ramp
            for _ in range(10):
                wdst = ps.tile([P, QCH], F32, tag="fill", bufs=2, name="wdst")
                nc.tensor.matmul(wdst[:], ident[:], wsrc[:],
                                 start=True, stop=True)

            # ---- AllToAll plumbing: one exchange per (batch, half) ----
            # Exchange e=(b,hf) covers b's tokens [hf*1024, hf*1024+1024) in
            # 8 blocks of TOKB=128. in[e][s] = my 128 feature rows of block
            # s; out[e][s] = rank s's 128 feature rows of MY block.
            TOKB = T // 2 // N_CORES  # 128
            a2a_in = {(b, hf): dram.tile([N_CORES, P, TOKB], BF,
                                         name=f"a2a_in{b}{hf}")
                      for b in range(B) for hf in range(2)}
            a2a_out = {(b, hf): dram.tile([N_CORES, P, TOKB], BF,
                                          name=f"a2a_out{b}{hf}")
                       for b in range(B) for hf in range(2)}

            def proj_group(tch, wsb, dst):
                tsl = slice(tch * QCH, (tch + 1) * QCH)
                pj = ps.tile([P, QCH], F32, tag="fill", bufs=2, name="pj")
                for ci in range(NCH):
                    nc.tensor.matmul(
                        pj[:], wsb[:, ci, :], xT_sb[:, ci, tsl],
                        start=(ci == 0), stop=(ci == NCH - 1),
                    )
                nc.vector.tensor_copy(dst[:, tsl], pj[:])

            def vtrans(t32):
                tr = ps.tile([P, P], BF, tag="fill", bufs=2, name="tr")
                nc.tensor.transpose(
                    tr[:], vT_sb[:, t32 * P:(t32 + 1) * P], ident[:]
                )
                out_ap = v_sb[:, t32, :].rearrange(
                    "p (h x) -> p h x", h=HL
                )[:, :, 0:D]
                in_ap = tr[:].rearrange("p (h x) -> p h x", h=HL)
                nc.vector.tensor_copy(out_ap, in_ap)

            ya_tiles = {}
            ob_tiles = {}

            def ya_load(b, hf):
                if b not in ya_tiles:
                    ya_tiles[b] = sb.tile([P, NCH, TTOK], BF, tag="ya",
                                          bufs=2, name="ya")
                    ob_tiles[b] = sb.tile([P, NCH, TTOK], BF, tag="ob",
                                          bufs=2, name="ob")
                nc.sync.dma_start(
                    ya_tiles[b][:, :, hf * TOKB:(hf + 1) * TOKB],
                    a2a_out[(b, hf)][:].rearrange("a p t -> p a t"),
                )

            def po_ro(b, ro, hf):
                # half-split so the half-0 matmuls can run while the second
                # A2A of the batch is still in flight
                ya = ya_tiles[b]
                tsl = slice(hf * TOKB, (hf + 1) * TOKB)
                po = ps.tile([P, TOKB], F32, tag="fill", bufs=2, name="po")
                for ci in range(NCH):
                    nc.tensor.matmul(
                        po[:], wo_sb[:, ci, ro, :], ya[:, ci, tsl],
                        start=(ci == 0), stop=(ci == NCH - 1),
                    )
                ob = ob_tiles[b]
                nc.vector.tensor_copy(ob[:, ro, tsl], po[:])
                if hf == 1:
                    # per-ro output write overlaps the next ro's matmuls
                    ov = out[:].rearrange("(r p) t -> p r t", p=P)
                    nc.sync.dma_start(
                        ov[:, ro, b * TTOK:(b + 1) * TTOK], ob[:, ro, :]
                    )

            # ---- prologue: projections for tch0 only ----
            for wsb, dst in ((wq_sb, qT_sb), (wk_sb, kT_sb), (wv_sb, vT_sb)):
                proj_group(0, wsb, dst)
            for t32 in range(4):
                vtrans(t32)

            # ---- the 80 attention pair-units ----
            units = []
            for b in range(B):
                for jq in range(NQC):
                    nkt = 4 * jq + 4
                    for h in range(HL):
                        for pr in range(nkt // 2):
                            units.append((b, jq, h, pr, nkt))
            NU = len(units)

            # filler schedule: unit index -> list of thunks
            SCHED = {}

            def at(u, fn, *args):
                SCHED.setdefault(u, []).append((fn, args))

            # projections tch1..7 (tch_k needed by (b=k//4, jq=k%4))
            proj_slots = {1: 0, 2: 4, 3: 12, 4: 20, 5: 27, 6: 33, 7: 44}
            for tch, u0 in proj_slots.items():
                for k, (wsb, dst) in enumerate(((wq_sb, qT_sb),
                                               (wk_sb, kT_sb),
                                               (wv_sb, vT_sb))):
                    at(u0 + k, proj_group, tch, wsb, dst)
                at(u0 + 3, vtrans, tch * 4 + 0)
                at(u0 + 3, vtrans, tch * 4 + 1)
                at(u0 + 4, vtrans, tch * 4 + 2)
                at(u0 + 4, vtrans, tch * 4 + 3)

            # b0's A2As fire at units 11 and 39; b1's first at unit 51.
            # Consumers are scheduled LATE: the first exchange can't finish
            # before the ~60us ncfw barrier, and inter-core skew (measured
            # up to ~40us) delays completion further — a too-early consumer
            # stalls this core's whole in-order PE queue.
            at(46, ya_load, 0, 0)
            for ro in range(NCH):
                at(56 + ro, po_ro, 0, ro, 0)
            at(64, ya_load, 0, 1)
            for ro in range(NCH):
                at(68 + ro, po_ro, 0, ro, 1)
            at(72, ya_load, 1, 0)

            st_of = {}
            pt_of = {}
            yt_of = {}
            den_of = {}

            def sc_issue(i):
                b, jq, h, pr, nkt = units[i]
                rsl = slice(h * D, (h + 1) * D)
                q0 = b * T + jq * QCH
                st = ps.tile([P, 2 * QCH], F32, tag="st", bufs=2, name="st")
                st_of[i] = st
                diag = []
                for half in range(2):
                    kt = 2 * pr + half
                    k0 = b * T + kt * P
                    ii = kt - 4 * jq
                    qv = max(ii, 0) * P
                    ssl = slice(half * QCH + qv, (half + 1) * QCH)
                    nc.tensor.matmul(
                        st[:, ssl],
                        kT_sb[rsl, k0:k0 + P],
                        qT_sb[rsl, q0 + qv:q0 + QCH],
                        start=True, stop=(ii < 0),
                    )
                    if ii >= 0:
                        diag.append(half * QCH + qv)
                for cq in diag:
                    nc.tensor.matmul(
                        st[:, cq:cq + P], ident[:], mneg[:],
                        start=False, stop=True,
                    )

            def exp_issue(i):
                b, jq, h, pr, nkt = units[i]
                qv0 = max(2 * pr - 4 * jq, 0) * P
                pt = sb.tile([P, 2 * QCH], BF, tag="pt", bufs=4, name="pt")
                pt_of[i] = pt
                nc.scalar.activation(
                    pt[:, qv0:], st_of[i][:, qv0:], AF.Exp,
                    scale=float(SCALE)
                )

            def pv_issue(i):
                b, jq, h, pr, nkt = units[i]
                if pr == 0:
                    yt_of[(b, jq, h)] = ps.tile([D + 1, QCH], F32, tag="yt",
                                                bufs=2, name="yt")
                yt = yt_of[(b, jq, h)]
                pt = pt_of.pop(i)
                for half in range(2):
                    kt = 2 * pr + half
                    qv = max(kt - 4 * jq, 0) * P
                    nc.tensor.matmul(
                        yt[:, qv:QCH],
                        v_sb[:, b * NKT + kt,
                             h * (D + 1):(h + 1) * (D + 1)],
                        pt[:, half * QCH + qv:(half + 1) * QCH],
                        start=(kt == 0), stop=(kt == nkt - 1),
                    )
                st_of.pop(i)
                if pr == nkt // 2 - 1:
                    den = sb.tile([1, QCH], F32, tag="den", bufs=4,
                                  name="den")
                    nc.vector.tensor_copy(den[:], yt[D:D + 1, :])
                    den_of[(b, jq, h)] = den

            def evict(b, jq, h):
                yt = yt_of.pop((b, jq, h))
                den = den_of.pop((b, jq, h))
                last = (b, jq, h) == (B - 1, NQC - 1, HL - 1)
                if last:
                    # PE is idle after the final pv: a K=1 ones-matmul
                    # broadcast is ~1.5us faster than the gpsimd hop here
                    bc = ps.tile([D, QCH], F32, tag="fill", bufs=2,
                                 name="bcp")
                    nc.tensor.matmul(bc[:], ones_col[:], den[:],
                                     start=True, stop=True)
                else:
                    # mid-stream the broadcast must stay OFF the in-order
                    # PE queue (a PE-side wait on the DVE den copy would
                    # stall the attention pipeline at every eviction)
                    bc = sb.tile([D, QCH], F32, tag="bc", bufs=2, name="bc")
                    nc.gpsimd.partition_broadcast(bc[:], den[:])
                rcp = sb.tile([D, QCH], F32, tag="rcp", bufs=2, name="rcp")
                nc.vector.reciprocal_approx_fast(rcp[:], bc[:])
                yn = sb.tile([D, QCH], BF, tag="yn", bufs=4, name="yn")
                nc.vector.tensor_mul(yn[:], yt[0:D, :], rcp[:])
                # tokens [jq*512, jq*512+512) = blocks [4*(jq%2), +4) of
                # exchange (b, jq//2). SBUF side keeps partition dim first;
                # the DRAM side view is reordered to match.
                hf = jq // 2
                s0 = 4 * (jq % 2)
                nc.gpsimd.dma_start(
                    a2a_in[(b, hf)][s0:s0 + 4, h * D:(h + 1) * D, :]
                    .rearrange("s d t -> d s t"),
                    yn[:].rearrange("d (s t) -> d s t", s=4),
                )
                if jq % 2 == 1 and h == HL - 1:
                    nc.gpsimd.collective_compute(
                        "AllToAll", mybir.AluOpType.bypass,
                        replica_groups=[list(range(N_CORES))],
                        ins=[a2a_in[(b, hf)][:]], outs=[a2a_out[(b, hf)][:]],
                    )
                return yn

            # ---- pipelined emission ----
            sc_issue(0)
            exp_issue(0)
            last_yn = None
            for i in range(NU):
                if i + 1 < NU:
                    sc_issue(i + 1)
                    exp_issue(i + 1)
                pv_issue(i)
                b, jq, h, pr, nkt = units[i]
                if pr == nkt // 2 - 1:
                    last_yn = evict(b, jq, h)
                for fn, args in SCHED.get(i, ()):
                    fn(*args)

            # ---- tail: b1's last A2A + O-projection ----
            # the half-0 projections (data arrived with b1's first A2A)
            # bridge the final A2A's latency with useful PE work; a few
            # anchored warm matmuls (reading the last yn so Tile can't
            # hoist them) cover the rest
            ya_load(1, 1)
            for ro in range(NCH):
                po_ro(1, ro, 0)
            for _ in range(6):
                wdst = ps.tile([P, QCH], F32, tag="fill", bufs=2, name="wdst")
                nc.tensor.matmul(wdst[:], ident[0:D, :], last_yn[:],
                                 start=True, stop=True)
            for ro in range(NCH):
                po_ro(1, ro, 1)

    nc.finalize()
    return nc


_GRAPH = None


def _get_graph():
    global _GRAPH
    if _GRAPH is None:
        _GRAPH = build_graph()
    return _GRAPH


def prepare_in_maps(x, Wq, Wk, Wv, Wo):
    x = np.asarray(x, np.float32)
    Wq = np.asarray(Wq, np.float32)
    Wk = np.asarray(Wk, np.float32)
    Wv = np.asarray(Wv, np.float32)
    Wo = np.asarray(Wo, np.float32)

    bf = ml_dtypes.bfloat16
    xTh = np.ascontiguousarray(x.reshape(TT, C).T).astype(bf)
    # full Wo^T packed as [p, ci, ro, m] = Wo.T[ci*128+p, ro*128+m]
    wo_pack = Wo.T.reshape(NCH, P, NCH, P).transpose(1, 0, 2, 3)
    in_maps = []
    for r in range(N_CORES):
        sl = slice(r * DL, (r + 1) * DL)
        # pack the 3 transposed qkv weight shards into the SBUF layout
        # [p, w, ci, m] where the shard row index is c = ci*128 + p
        wqkv = np.empty((P, 3, NCH, DL), np.float32)
        for w, W in enumerate((Wq, Wk, Wv)):
            wqkv[:, w] = W[sl].T.reshape(NCH, P, DL).transpose(1, 0, 2)
        wall = np.concatenate(
            [wqkv.reshape(P, W_QKV), wo_pack.reshape(P, W_O)], axis=1
        )
        in_maps.append({
            "xT": xTh,
            "wall": np.ascontiguousarray(wall).astype(bf),
        })
    return in_maps


def assemble_output(results):
    TOKB = T // 2 // N_CORES
    outT = np.empty((C, TT), np.float32)
    for r in range(N_CORES):
        o = np.asarray(results[r]["out"], np.float32)  # [C, 2*TTOK]
        for b in range(B):
            ob = o[:, b * TTOK:(b + 1) * TTOK]
            for hf in range(2):
                t0 = b * T + hf * (T // 2) + r * TOKB
                outT[:, t0:t0 + TOKB] = ob[:, hf * TOKB:(hf + 1) * TOKB]
    return np.ascontiguousarray(outT.T).reshape(B, T, C)


def kernel(x, Wq, Wk, Wv, Wo):
    nc = _get_graph()
    in_maps = prepare_in_maps(x, Wq, Wk, Wv, Wo)
    res = run_bass_kernel_spmd(nc, in_maps, core_ids=list(range(N_CORES)))
    return assemble_output(res.results)


# revision 23
# speedup vs baseline: 1.5757x; 1.2143x over previous
"""Causal multi-head attention block (B=2, T=2048, C=1024, H=16) on 8 TRN2
NeuronCores.

Sharding: Megatron-style tensor parallel over heads for QKV + attention
(core r owns heads {2r, 2r+1} = feature rows [128r, 128r+128) of the
attention output), then a token-sharded output projection: cores
exchange attention outputs with ONE AllToAll per batch entry (each core
sends its 128 feature rows of everyone's 256-token block, receives the
full 1024 feature rows of its own 256-token block), and each core
computes out[:, its tokens] = Wo @ y_full with the full Wo replicated.

v2 used Megatron all-gather + column-sharded O-projection: that moves
8 MB per core through the collective stream (AllGather replicates), and
the profiled CC stream (45us first-collective barrier + 8 x ~20us
sequential RDH AllGathers at ~50 GB/s) was the critical path of the
whole second half. The AllToAll moves only ~0.5 MB per core per batch
(mesh algorithm, ~10us) and there are just two of them.

Everything on-device is computed in the "transposed" orientation
(feature-major, token-minor) so the TensorEngine contraction axis always
sits on SBUF partitions and the softmax denominator arrives for free via
a ones-column appended to V:

  qT/kT/vT [128, 4096] = W_shard @ x^T          (x^T passed from host)
  ST tile [128k, 512q] = kT_slice.T @ qT_slice  (contract d=64)
  causal mask: add a -1e9 strictly-lower-triangular matrix into the St
      PSUM accumulation group via matmul(ident, mneg) on diagonal blocks
  PT = exp(ST * 1/sqrt(d))                      (no max-subtraction: logits
                                                 are ~N(0,1), |S|max ~ 6)
  yT [65, 512] += [v | 1].T @ PT                (row 64 = softmax denom)
  yT_norm = yT[0:64] * partition_broadcast(recip(yT[64]))

Performance structure (from perfetto/HAM/cc_ops analysis of v1/v2):
- The attention is a flat list of 80 "pair units" (2 k-tiles sharing a
  2-bank PSUM st tile, one 1024-wide EXP each), software-pipelined:
  scores(i+1) are program-ordered BEFORE pv(i) so the in-order PE queue
  never waits on the ACT exp of the current pair.
- All projections except tch0, and the b0 O-projection, are filler
  thunks injected between units at fixed slots, keeping the PE stream
  dense (HAM stays un-throttled) while ACT runs exps back to back.
- A tiny dummy AllGather fires at t~0 so the ~45us one-time ncfw
  barrier (paid at the first collective) overlaps the prologue instead
  of the first real collective consumer.
- xT is loaded with 4 big dma_starts (each fans out over all 16 DMA
  engines) on 3 queues; the scalar queue only carries the 80 exps plus
  the tch0 load, so DMA issue never delays them. The exp table set is
  pre-loaded with a dummy activation at t=0.

Inputs are bf16 (host-side cast); accumulation is f32 in PSUM; the output
shard is written bf16 and upcast to f32 on the host.
"""

import numpy as np
import ml_dtypes

import concourse.bacc as bacc
import concourse.mybir as mybir
import concourse.tile as tile
from concourse.bass_utils import run_bass_kernel_spmd
from concourse.masks import make_identity

N_CORES = 8
B, T, C, H = 2, 2048, 1024, 16
D = 64                # head dim
HL = H // N_CORES     # heads per core = 2
DL = HL * D           # local feature dim = 128
TT = B * T            # 4096 tokens total
P = 128
NCH = C // P          # 8 contraction chunks
QCH = 512             # q-chunk (moving free dim)
NQC = T // QCH        # 4 q-chunks per batch entry
NKT = T // P          # 16 k-tiles per batch entry
TTOK = T // N_CORES   # 256: tokens per core per batch in the A2A split
SCALE = 1.0 / np.sqrt(D)

BF = mybir.dt.bfloat16
F32 = mybir.dt.float32
AF = mybir.ActivationFunctionType

W_QKV = 3 * NCH * DL          # 3072 cols of packed qkv shards
W_O = NCH * C                 # 8192 cols of packed full Wo


def build_graph():
    nc = bacc.Bacc("TRN2", target_bir_lowering=False, debug=False)

    xT = nc.dram_tensor("xT", [C, TT], BF, kind="ExternalInput")
    wall = nc.dram_tensor("wall", [P, W_QKV + W_O], BF, kind="ExternalInput")
    # out[:, 0:256] = batch-0 tokens [256r, 256r+256); [:, 256:512] same
    # for batch 1
    out = nc.dram_tensor("out", [C, 2 * TTOK], BF, kind="ExternalOutput")

    with tile.TileContext(nc) as tc:
        with (
            tc.tile_pool(name="sb", bufs=1) as sb,
            tc.tile_pool(name="ps", bufs=1, space="PSUM") as ps,
            tc.tile_pool(name="dram", bufs=1, space="DRAM") as dram,
        ):
            # ---- collective warm-up: absorb the one-time ncfw barrier and
            # the first-AllToAll setup cost ----
            ccw_in = dram.tile([N_CORES, 32], BF, name="ccw_in")
            ccw_out = dram.tile([N_CORES, 32], BF, name="ccw_out")
            nc.gpsimd.collective_compute(
                "AllToAll", mybir.AluOpType.bypass,
                replica_groups=[list(range(N_CORES))],
                ins=[ccw_in[:]], outs=[ccw_out[:]],
            )

            # ---- loads ----
            # xT chunk [512:1024] leads the gpsimd queue so tch1 doesn't
            # serialize behind the big weight transfer on sync
            xT_sb = sb.tile([P, NCH, TT], BF, name="xT_sb")
            xTr = xT[:].rearrange("(a p) t -> p a t", p=P)
            nc.gpsimd.dma_start(xT_sb[:, :, QCH:2 * QCH],
                                xTr[:, :, QCH:2 * QCH])
            nc.gpsimd.dma_start(xT_sb[:, :, 4 * QCH:TT], xTr[:, :, 4 * QCH:TT])

            w_sb = sb.tile([P, W_QKV + W_O], BF, name="w_sb")
            nc.sync.dma_start(w_sb[:, 0:W_QKV], wall[:, 0:W_QKV])
            w3 = w_sb[:, 0:W_QKV].rearrange("p (w a m) -> p w a m",
                                            w=3, a=NCH)
            wq_sb, wk_sb, wv_sb = (w3[:, i] for i in range(3))
            # full Wo^T packed as [p, ci, ro, m]:
            # Wo.T[ci*128+p, ro*128+m]
            wo_sb = w_sb[:, W_QKV:].rearrange("p (a r m) -> p a r m",
                                              a=NCH, r=NCH)

            ident = sb.tile([P, P], BF, name="ident")
            make_identity(nc, ident)
            # strictly-lower-triangular -1e9 (k > q): masks causal logits on
            # diagonal blocks, injected into the St PSUM group via
            # matmul(ident, mneg)
            mneg = sb.tile([P, P], BF, name="mneg")
            nc.gpsimd.memset(mneg[:], 0.0)
            nc.gpsimd.affine_select(
                out=mneg[:], in_=mneg[:],
                compare_op=mybir.AluOpType.is_ge,
                fill=-1e9, base=0, channel_multiplier=-1, pattern=[[1, P]],
            )
            wsrc = sb.tile([P, QCH], BF, name="wsrc")
            nc.vector.memset(wsrc[:], 0.5)
            # K=1 stationary of ones: broadcasts the denominator row across
            # 64 partitions on the (otherwise idle-at-that-moment) PE
            ones_col = sb.tile([1, D], F32, name="ones_col")
            nc.vector.memset(ones_col[:], 1.0)
            # pre-load the exp table set so the first real exp doesn't pay
            # the ~2.7us ACT_TABLE_LOAD on the critical path
            expwarm = sb.tile([P, 32], F32, name="expwarm")
            nc.scalar.activation(expwarm[:], wsrc[:, 0:32], AF.Exp, scale=1.0)

            # remaining xT + the big Wo block (needed only mid-kernel)
            nc.scalar.dma_start(xT_sb[:, :, 0:QCH], xTr[:, :, 0:QCH])
            nc.sync.dma_start(xT_sb[:, :, 2 * QCH:4 * QCH],
                              xTr[:, :, 2 * QCH:4 * QCH])
            nc.sync.dma_start(w_sb[:, W_QKV:], wall[:, W_QKV:])

            qT_sb = sb.tile([P, TT], BF, name="qT_sb")
            kT_sb = sb.tile([P, TT], BF, name="kT_sb")
            vT_sb = sb.tile([P, TT], BF, name="vT_sb")
            # v in natural layout, packed per 128-token tile as
            # [headA(64) | 1 | headB(64) | 1] -> 130 columns
            v_sb = sb.tile([P, TT // P, 2 * (D + 1)], BF, name="v_sb")
            nc.gpsimd.memset(v_sb[:], 1.0)

            # warmup matmuls: cover the initial DMA window + HAM ramp
            for _ in range(10):
                wdst = ps.tile([P, QCH], F32, tag="fill", bufs=2, name="wdst")
                nc.tensor.matmul(wdst[:], ident[:], wsrc[:],
                                 start=True, stop=True)

            # ---- AllToAll plumbing: one exchange per (batch, half) ----
            # Exchange e=(b,hf) covers b's tokens [hf*1024, hf*1024+1024) in
            # 8 blocks of TOKB=128. in[e][s] = my 128 feature rows of block
            # s; out[e][s] = rank s's 128 feature rows of MY block.
            TOKB = T // 2 // N_CORES  # 128
            a2a_in = {(b, hf): dram.tile([N_CORES, P, TOKB], BF,
                                         name=f"a2a_in{b}{hf}")
                      for b in range(B) for hf in range(2)}
            a2a_out = {(b, hf): dram.tile([N_CORES, P, TOKB], BF,
                                          name=f"a2a_out{b}{hf}")
                       for b in range(B) for hf in range(2)}

            def proj_group(tch, wsb, dst):
                tsl = slice(tch * QCH, (tch + 1) * QCH)
                pj = ps.tile([P, QCH], F32, tag="fill", bufs=2, name="pj")
                for ci in range(NCH):
                    nc.tensor.matmul(
                        pj[:], wsb[:, ci, :], xT_sb[:, ci, tsl],
                        start=(ci == 0), stop=(ci == NCH - 1),
                    )
                nc.vector.tensor_copy(dst[:, tsl], pj[:])

            def vtrans(t32):
                tr = ps.tile([P, P], BF, tag="fill", bufs=2, name="tr")
                nc.tensor.transpose(
                    tr[:], vT_sb[:, t32 * P:(t32 + 1) * P], ident[:]
                )
                out_ap = v_sb[:, t32, :].rearrange(
                    "p (h x) -> p h x", h=HL
                )[:, :, 0:D]
                in_ap = tr[:].rearrange("p (h x) -> p h x", h=HL)
                nc.vector.tensor_copy(out_ap, in_ap)

            ya_tiles = {}
            ob_tiles = {}

            def ya_load(b, hf):
                if b not in ya_tiles:
                    ya_tiles[b] = sb.tile([P, NCH, TTOK], BF, tag="ya",
                                          bufs=2, name="ya")
                    ob_tiles[b] = sb.tile([P, NCH, TTOK], BF, tag="ob",
                                          bufs=2, name="ob")
                nc.sync.dma_start(
                    ya_tiles[b][:, :, hf * TOKB:(hf + 1) * TOKB],
                    a2a_out[(b, hf)][:].rearrange("a p t -> p a t"),
                )

            def po_ro(b, ro, hf):
                # half-split so the half-0 matmuls can run while the second
                # A2A of the batch is still in flight
                ya = ya_tiles[b]
                tsl = slice(hf * TOKB, (hf + 1) * TOKB)
                po = ps.tile([P, TOKB], F32, tag="fill", bufs=2, name="po")
                for ci in range(NCH):
                    nc.tensor.matmul(
                        po[:], wo_sb[:, ci, ro, :], ya[:, ci, tsl],
                        start=(ci == 0), stop=(ci == NCH - 1),
                    )
                ob = ob_tiles[b]
                nc.vector.tensor_copy(ob[:, ro, tsl], po[:])
                if hf == 1:
                    # per-ro output write overlaps the next ro's matmuls
                    ov = out[:].rearrange("(r p) t -> p r t", p=P)
                    nc.sync.dma_start(
                        ov[:, ro, b * TTOK:(b + 1) * TTOK], ob[:, ro, :]
                    )

            # ---- prologue: projections for tch0 only ----
            for wsb, dst in ((wq_sb, qT_sb), (wk_sb, kT_sb), (wv_sb, vT_sb)):
                proj_group(0, wsb, dst)
            for t32 in range(4):
                vtrans(t32)

            # ---- the 80 attention pair-units ----
            units = []
            for b in range(B):
                for jq in range(NQC):
                    nkt = 4 * jq + 4
                    for h in range(HL):
                        for pr in range(nkt // 2):
                            units.append((b, jq, h, pr, nkt))
            NU = len(units)

            # filler schedule: unit index -> list of thunks
            SCHED = {}

            def at(u, fn, *args):
                SCHED.setdefault(u, []).append((fn, args))

            # projections tch1..7 (tch_k needed by (b=k//4, jq=k%4))
            proj_slots = {1: 0, 2: 4, 3: 12, 4: 20, 5: 27, 6: 33, 7: 44}
            for tch, u0 in proj_slots.items():
                for k, (wsb, dst) in enumerate(((wq_sb, qT_sb),
                                               (wk_sb, kT_sb),
                                               (wv_sb, vT_sb))):
                    at(u0 + k, proj_group, tch, wsb, dst)
                at(u0 + 3, vtrans, tch * 4 + 0)
                at(u0 + 3, vtrans, tch * 4 + 1)
                at(u0 + 4, vtrans, tch * 4 + 2)
                at(u0 + 4, vtrans, tch * 4 + 3)

            # NOTE: no exchange consumers are scheduled inside the attention
            # stream. The ncfw barrier (59-108us, variable) plus inter-core
            # skew make exchange completion times unpredictable; any
            # fixed-slot consumer stalls this core's in-order PE queue and
            # the stall cascades across cores through later exchanges. All
            # O-projection work runs in the tail, where exchanges 1-3 are
            # long done and po(b0) + po(b1,half0) usefully bridge the final
            # A2A's flight time.

            st_of = {}
            pt_of = {}
            yt_of = {}
            den_of = {}

            def sc_issue(i):
                b, jq, h, pr, nkt = units[i]
                rsl = slice(h * D, (h + 1) * D)
                q0 = b * T + jq * QCH
                st = ps.tile([P, 2 * QCH], F32, tag="st", bufs=2, name="st")
                st_of[i] = st
                diag = []
                for half in range(2):
                    kt = 2 * pr + half
                    k0 = b * T + kt * P
                    ii = kt - 4 * jq
                    qv = max(ii, 0) * P
                    ssl = slice(half * QCH + qv, (half + 1) * QCH)
                    nc.tensor.matmul(
                        st[:, ssl],
                        kT_sb[rsl, k0:k0 + P],
                        qT_sb[rsl, q0 + qv:q0 + QCH],
                        start=True, stop=(ii < 0),
                    )
                    if ii >= 0:
                        diag.append(half * QCH + qv)
                for cq in diag:
                    nc.tensor.matmul(
                        st[:, cq:cq + P], ident[:], mneg[:],
                        start=False, stop=True,
                    )

            def exp_issue(i):
                b, jq, h, pr, nkt = units[i]
                qv0 = max(2 * pr - 4 * jq, 0) * P
                pt = sb.tile([P, 2 * QCH], BF, tag="pt", bufs=4, name="pt")
                pt_of[i] = pt
                nc.scalar.activation(
                    pt[:, qv0:], st_of[i][:, qv0:], AF.Exp,
                    scale=float(SCALE)
                )

            def pv_issue(i):
                b, jq, h, pr, nkt = units[i]
                if pr == 0:
                    yt_of[(b, jq, h)] = ps.tile([D + 1, QCH], F32, tag="yt",
                                                bufs=2, name="yt")
                yt = yt_of[(b, jq, h)]
                pt = pt_of.pop(i)
                for half in range(2):
                    kt = 2 * pr + half
                    qv = max(kt - 4 * jq, 0) * P
                    nc.tensor.matmul(
                        yt[:, qv:QCH],
                        v_sb[:, b * NKT + kt,
                             h * (D + 1):(h + 1) * (D + 1)],
                        pt[:, half * QCH + qv:(half + 1) * QCH],
                        start=(kt == 0), stop=(kt == nkt - 1),
                    )
                st_of.pop(i)
                if pr == nkt // 2 - 1:
                    den = sb.tile([1, QCH], F32, tag="den", bufs=4,
                                  name="den")
                    nc.vector.tensor_copy(den[:], yt[D:D + 1, :])
                    den_of[(b, jq, h)] = den

            def evict(b, jq, h):
                yt = yt_of.pop((b, jq, h))
                den = den_of.pop((b, jq, h))
                last = (b, jq, h) == (B - 1, NQC - 1, HL - 1)
                if last:
                    # PE is idle after the final pv: a K=1 ones-matmul
                    # broadcast is ~1.5us faster than the gpsimd hop here
                    bc = ps.tile([D, QCH], F32, tag="fill", bufs=2,
                                 name="bcp")
                    nc.tensor.matmul(bc[:], ones_col[:], den[:],
                                     start=True, stop=True)
                else:
                    # mid-stream the broadcast must stay OFF the in-order
                    # PE queue (a PE-side wait on the DVE den copy would
                    # stall the attention pipeline at every eviction)
                    bc = sb.tile([D, QCH], F32, tag="bc", bufs=2, name="bc")
                    nc.gpsimd.partition_broadcast(bc[:], den[:])
                rcp = sb.tile([D, QCH], F32, tag="rcp", bufs=2, name="rcp")
                nc.vector.reciprocal_approx_fast(rcp[:], bc[:])
                yn = sb.tile([D, QCH], BF, tag="yn", bufs=4, name="yn")
                nc.vector.tensor_mul(yn[:], yt[0:D, :], rcp[:])
                # tokens [jq*512, jq*512+512) = blocks [4*(jq%2), +4) of
                # exchange (b, jq//2). SBUF side keeps partition dim first;
                # the DRAM side view is reordered to match.
                hf = jq // 2
                s0 = 4 * (jq % 2)
                nc.gpsimd.dma_start(
                    a2a_in[(b, hf)][s0:s0 + 4, h * D:(h + 1) * D, :]
                    .rearrange("s d t -> d s t"),
                    yn[:].rearrange("d (s t) -> d s t", s=4),
                )
                if jq % 2 == 1 and h == HL - 1:
                    nc.gpsimd.collective_compute(
                        "AllToAll", mybir.AluOpType.bypass,
                        replica_groups=[list(range(N_CORES))],
                        ins=[a2a_in[(b, hf)][:]], outs=[a2a_out[(b, hf)][:]],
                    )
                return yn

            # ---- pipelined emission ----
            sc_issue(0)
            exp_issue(0)
            last_yn = None
            for i in range(NU):
                if i + 1 < NU:
                    sc_issue(i + 1)
                    exp_issue(i + 1)
                pv_issue(i)
                b, jq, h, pr, nkt = units[i]
                if pr == nkt // 2 - 1:
                    last_yn = evict(b, jq, h)
                for fn, args in SCHED.get(i, ()):
                    fn(*args)

            # ---- tail: all O-projection work ----
            # the last A2A was fired inside the final evict; everything
            # except po(b1, half1) has its data ready and bridges its flight
            del last_yn
            ya_load(0, 0)
            ya_load(0, 1)
            ya_load(1, 0)
            for ro in range(NCH):
                po_ro(0, ro, 0)
                po_ro(0, ro, 1)
            for ro in range(NCH):
                po_ro(1, ro, 0)
            ya_load(1, 1)
            for ro in range(NCH):
                po_ro(1, ro, 1)

    nc.finalize()
    return nc


_GRAPH = None


def _get_graph():
    global _GRAPH
    if _GRAPH is None:
        _GRAPH = build_graph()
    return _GRAPH


def prepare_in_maps(x, Wq, Wk, Wv, Wo):
    x = np.asarray(x, np.float32)
    Wq = np.asarray(Wq, np.float32)
    Wk = np.asarray(Wk, np.float32)
    Wv = np.asarray(Wv, np.float32)
    Wo = np.asarray(Wo, np.float32)

    bf = ml_dtypes.bfloat16
    xTh = np.ascontiguousarray(x.reshape(TT, C).T).astype(bf)
    # full Wo^T packed as [p, ci, ro, m] = Wo.T[ci*128+p, ro*128+m]
    wo_pack = Wo.T.reshape(NCH, P, NCH, P).transpose(1, 0, 2, 3)
    in_maps = []
    for r in range(N_CORES):
        sl = slice(r * DL, (r + 1) * DL)
        # pack the 3 transposed qkv weight shards into the SBUF layout
        # [p, w, ci, m] where the shard row index is c = ci*128 + p
        wqkv = np.empty((P, 3, NCH, DL), np.float32)
        for w, W in enumerate((Wq, Wk, Wv)):
            wqkv[:, w] = W[sl].T.reshape(NCH, P, DL).transpose(1, 0, 2)
        wall = np.concatenate(
            [wqkv.reshape(P, W_QKV), wo_pack.reshape(P, W_O)], axis=1
        )
        in_maps.append({
            "xT": xTh,
            "wall": np.ascontiguousarray(wall).astype(bf),
        })
    return in_maps


def assemble_output(results):
    TOKB = T // 2 // N_CORES
    outT = np.empty((C, TT), np.float32)
    for r in range(N_CORES):
        o = np.asarray(results[r]["out"], np.float32)  # [C, 2*TTOK]
        for b in range(B):
            ob = o[:, b * TTOK:(b + 1) * TTOK]
            for hf in range(2):
                t0 = b * T + hf * (T // 2) + r * TOKB
                outT[:, t0:t0 + TOKB] = ob[:, hf * TOKB:(hf + 1) * TOKB]
    return np.ascontiguousarray(outT.T).reshape(B, T, C)


def kernel(x, Wq, Wk, Wv, Wo):
    nc = _get_graph()
    in_maps = prepare_in_maps(x, Wq, Wk, Wv, Wo)
    res = run_bass_kernel_spmd(nc, in_maps, core_ids=list(range(N_CORES)))
    return assemble_output(res.results)
